# revision 1
# baseline (speedup 1.0000x reference)
"""DRConv (dynamic region-aware conv) Trainium2 kernel.

Math (per batch b, all on device):
  x_se  = 0.25*sigmoid(routing_w @ mean_hw(x) + routing_b)           # [G*T]
  Z_t   = conv3x3(x, template_t)       for t in 0..T-1               # [O, H, W]
  U     = [x_se.T | 1] contracted with exp(Alpha) over g             # [T+1, P]
  out   = (sum_t Z_t * U_t) / U_T  + bias                            # [O, H, W]
which equals the reference
  out = einsum('boghw,bghw->bohw', einsum('bokg,bkhw->boghw', w, patches),
               softmax(Alpha)) + bias
because w = blend(x_se, templates) commutes through the conv: the blend
weights x_se[g,t] and the softmax probs both act per (g, pixel), so the
G-sum and T-sum exchange with the K-contraction.

Sharding: data-parallel over batch B=8, one batch element per NeuronCore.
Templates/routing weights replicated. No collectives.

Device layout (per core):
  pixels live in a 58x57 plane: one pad row top/bottom, ONE pad column
  (a right-pad column doubles as the left neighbor of the next row's
  x=0 pixel, so 57-wide rows give correct 3x3 zero padding);
  pf = (y+1)*57 + x for image pixel (y, x).
  conv = 9 shifted matmuls accumulating in PSUM:
    Z[px, (t,o)] += x[c, base+px+delta(i,j)].T @ tmpl[c, (t,o)]
  pixel tiles are the stationary operand (128 px per matmul), so the
  per-pixel softmax mixing becomes per-partition scalar_tensor_tensor ops,
  and the final [px, o] -> [o, px] flip is a PE transpose.
"""

import ml_dtypes
import numpy as np

import concourse.bass as bass
import concourse.mybir as mybir
from concourse import bacc, masks
from concourse.tile import TileContext
from concourse.bass_utils import run_bass_kernel_spmd

# problem constants
C = 128          # in channels
O = 128          # out channels
H = W = 56
G = 8            # groups
T = 8            # num weight templates
WP = 57          # padded row width (one shared pad column)
HPAD = 58        # one pad row top and bottom
NPIX = HPAD * WP  # 3306
GUARD = 64       # front guard in the x buffer for negative conv shifts
OFREE = 3328     # 26*128 >= NPIX
PT0 = WP         # first pixel-tile starts at padded row 1
NT = 25          # 25 tiles of 128 px cover pf [57, 3257) > last valid 3247
NCORES = 8

_cache = {}


def _delta(ij):
    i, j = divmod(ij, 3)
    return (i - 1) * WP + (j - 1)


def _build(use_alpha: int):
    f32 = mybir.dt.float32
    bf16 = mybir.dt.bfloat16

    nc = bacc.Bacc("TRN2", target_bir_lowering=False, debug=False,
                   num_devices=NCORES)

    # image ships as bf16 (matmuls are bf16 anyway), split in two row
    # bands so early pixel tiles only wait for the first band
    x0_d = nc.dram_tensor("x0", [C, 31 * W], bf16, kind="ExternalInput")
    x1_d = nc.dram_tensor("x1", [C, 28 * W], bf16, kind="ExternalInput")
    alpha_d = nc.dram_tensor("alpha", [G, H, W], f32, kind="ExternalInput")
    tmpl_d = nc.dram_tensor("tmpl", [9, C, T * O], bf16, kind="ExternalInput")
    rwt_d = nc.dram_tensor("rwt", [C, G * T], f32, kind="ExternalInput")
    rb_d = nc.dram_tensor("rb", [G * T], f32, kind="ExternalInput")
    bias_d = nc.dram_tensor("bias", [O], f32, kind="ExternalInput")
    mask_d = None
    if not use_alpha:
        mask_d = nc.dram_tensor("mask", [H, W], mybir.dt.int32,
                                kind="ExternalInput")
    out_d = nc.dram_tensor("out", [O, OFREE], f32, kind="ExternalOutput")

    with TileContext(nc) as tc:
        with (
            tc.tile_pool(name="big", bufs=1) as big,
            tc.tile_pool(name="consts", bufs=1) as consts,
            tc.tile_pool(name="stage", bufs=3) as stage,
            tc.tile_pool(name="acc", bufs=3) as accp,
            tc.tile_pool(name="upool", bufs=3) as upool,
            tc.tile_pool(name="zps", bufs=3, space="PSUM") as zps,
            tc.tile_pool(name="ups", bufs=1, space="PSUM") as ups,
            tc.tile_pool(name="tps", bufs=1, space="PSUM") as tps,
        ):
            # ---- constants ----
            ident = consts.tile([128, 128], f32)
            masks.make_identity(nc, ident[:])

            # PE warmup: dummy matmuls so HAM un-throttles while the
            # input DMAs stream in (needs only SBUF-resident data)
            warm = tps.tile([128, 128], f32, tag="tp", name="warm")
            for w_i in range(30):
                nc.tensor.matmul(warm[:], lhsT=ident[:], rhs=ident[:])

            bias_rep = consts.tile([128, O], f32)
            nc.sync.dma_start(
                out=bias_rep[:],
                in_=bass.AP(tensor=bias_d, offset=0, ap=[[0, 128], [1, O]]),
            )

            # ---- image band A + routing weights first ----
            xst0 = big.tile([C, 31 * W], bf16)
            nc.sync.dma_start(out=xst0[:], in_=x0_d[:])
            rwt = consts.tile([C, G * T], f32)
            nc.sync.dma_start(out=rwt[:], in_=rwt_d[:])
            rb = consts.tile([G * T, 1], f32)
            nc.sync.dma_start(out=rb[:], in_=rb_d[:])

            # band B DMA too (bf16 bands are small; land them both early)
            XB1 = 29 * WP                  # pf origin of band B buffer
            xst1 = big.tile([C, 28 * W], bf16)
            nc.sync.dma_start(out=xst1[:], in_=x1_d[:])

            # pixel tiles k<=12 read pf [-1, 1779) -> image rows 0..30
            xbf0 = big.tile([C, GUARD + 32 * WP], bf16)
            nc.vector.memset(xbf0[:], 0.0)
            v = xbf0[:, GUARD:GUARD + 32 * WP].rearrange(
                "c (h w) -> c h w", w=WP)
            nc.vector.tensor_copy(
                v[:, 1:32, 0:W], xst0[:].rearrange("c (h w) -> c h w", w=W))

            # ---- templates ----
            tbf = []
            for ij in range(9):
                tb = big.tile([C, T * O], bf16, name=f"tbf{ij}")
                nc.sync.dma_start(out=tb[:], in_=tmpl_d[ij])
                tbf.append(tb)

            # ---- image band B plane: k>=13 read pf [1663, 3315) ----
            xbf1 = big.tile([C, 30 * WP], bf16)
            nc.gpsimd.memset(xbf1[:], 0.0)
            v = xbf1[:, 0:30 * WP].rearrange("c (h w) -> c h w", w=WP)
            nc.gpsimd.tensor_copy(
                v[:, 0:28, 0:W], xst1[:].rearrange("c (h w) -> c h w", w=W))

            # ---- routing: GAP -> fc -> sigmoid (start ASAP) ----
            xsum = consts.tile([C, 1], f32)
            xsum0 = consts.tile([C, 1], f32)
            nc.vector.tensor_reduce(
                out=xsum0[:], in_=xst0[:],
                axis=mybir.AxisListType.X, op=mybir.AluOpType.add)
            nc.vector.tensor_reduce(
                out=xsum[:], in_=xst1[:, 3 * W:],
                axis=mybir.AxisListType.X, op=mybir.AluOpType.add)
            nc.vector.tensor_add(xsum[:], xsum[:], xsum0[:])

            zr = ups.tile([G * T, 1], f32, tag="up")
            nc.tensor.matmul(zr[:], lhsT=rwt[:], rhs=xsum[:])
            # x_se = (2/T)*sigmoid(fc(mean) + rb); mean folded into scale
            xse = consts.tile([G * T, 1], f32)
            nc.scalar.activation(xse[:], zr[:],
                                 mybir.ActivationFunctionType.Sigmoid,
                                 bias=rb[:], scale=1.0 / (H * W))
            xse4 = consts.tile([G * T, 1], bf16)
            nc.vector.tensor_scalar_mul(xse4[:], xse[:], 2.0 / T)

            # lhsT_U [g, T+1]: cols 0..T-1 = x_se[g, t], col T = 1.0
            # (the [64,1] -> [8,8] partition/free reshape is a tiny DMA)
            lhsu = consts.tile([G, T + 1], bf16)
            nc.vector.memset(lhsu[:, T:T + 1], 1.0)
            nc.sync.dma_start(out=lhsu[:, 0:T], in_=xse4[:])

            # ---- routing probability numerators ----
            ea = big.tile([G, OFREE], bf16)
            nc.gpsimd.memset(ea[:], 1.0)
            ea_core = ea[:, 0:NPIX].rearrange("g (h w) -> g h w", w=WP)
            if use_alpha:
                astage = stage.tile([G, H * W], f32, tag="astage")
                nc.sync.dma_start(out=astage[:], in_=alpha_d[:])
                nc.scalar.activation(
                    ea_core[:, 1:57, 0:W],
                    astage[:].rearrange("g (h w) -> g h w", w=W),
                    mybir.ActivationFunctionType.Exp)
            else:
                # hard routing: ea[g, p] = (mask[p] == g)
                mrow = stage.tile([1, H * W], mybir.dt.int32, tag="mrow")
                nc.sync.dma_start(out=mrow[:], in_=mask_d[:])
                mf = stage.tile([1, H * W], f32, tag="mf")
                nc.scalar.copy(mf[:], mrow[:])
                mrep = big.tile([G, H * W], f32)
                for g in range(G):
                    nc.sync.dma_start(out=mrep[g:g + 1, :], in_=mf[:])
                giota = consts.tile([G, 1], f32)
                for g in range(G):
                    nc.vector.memset(giota[g:g + 1, :], float(g))
                nc.vector.tensor_scalar(
                    ea_core[:, 1:57, 0:W],
                    mrep[:].rearrange("g (h w) -> g h w", w=W),
                    giota[:], None, op0=mybir.AluOpType.is_equal)

            # ---- output accumulation plane, 4 window-aligned chunks so
            # stores overlap compute and the tail only waits on the last ----
            OCUT = [0, PT0 + 128 * 7, PT0 + 128 * 13, PT0 + 128 * 19, OFREE]
            outsb = [big.tile([O, OCUT[i + 1] - OCUT[i]], f32,
                              name=f"outsb{i}") for i in range(4)]

            def outsb_slice(lo, n):
                for i in range(4):
                    if lo + n <= OCUT[i + 1]:
                        assert lo >= OCUT[i]
                        return outsb[i][:, lo - OCUT[i]:lo - OCUT[i] + n]
                raise AssertionError(lo)

            # ---- main loop over pixel tiles ----
            for k in range(NT):
                base = PT0 + 128 * k

                up = ups.tile([128, T + 1], f32, tag="up")
                nc.tensor.matmul(up[:], lhsT=ea[:, base:base + 128],
                                 rhs=lhsu[:])
                rcol = upool.tile([128, 1], f32, tag="rcol")
                nc.vector.reciprocal(rcol[:], up[:, T:T + 1])
                usb = upool.tile([128, T], f32, tag="usb")
                nc.vector.tensor_scalar_mul(usb[:], up[:, 0:T], rcol[:])

                zp = [zps.tile([128, 512], f32, tag=f"zp{h}",
                               name=f"zp{h}_{k}")
                      for h in range(2)]
                for ij in range(9):
                    if k <= 12:
                        lo = GUARD + base + _delta(ij)
                        xsl = xbf0[:, lo:lo + 128]
                    else:
                        lo = base - XB1 + _delta(ij)
                        xsl = xbf1[:, lo:lo + 128]
                    for h in range(2):
                        nc.tensor.matmul(
                            zp[h][:],
                            lhsT=xsl,
                            rhs=tbf[ij][:, h * 512:(h + 1) * 512],
                            start=(ij == 0), stop=(ij == 8))

                acc = accp.tile([128, O], f32, tag="acc")
                for t in range(T):
                    h, tq = divmod(t, 4)
                    nc.vector.scalar_tensor_tensor(
                        out=acc[:],
                        in0=zp[h][:, tq * 128:(tq + 1) * 128],
                        scalar=usb[:, t:t + 1],
                        in1=bias_rep[:] if t == 0 else acc[:],
                        op0=mybir.AluOpType.mult,
                        op1=mybir.AluOpType.add)

                tp = tps.tile([128, 128], f32, tag="tp")
                nc.tensor.transpose(tp[:], acc[:], ident[:])
                nc.scalar.copy(outsb_slice(base, 128), tp[:])

            # ---- store padded planes (host strips the padding) ----
            for i in range(4):
                nc.sync.dma_start(out=out_d[:, OCUT[i]:OCUT[i + 1]],
                                  in_=outsb[i][:])

    nc.compile()
    return nc


def _get(use_alpha: int):
    if use_alpha not in _cache:
        _cache[use_alpha] = _build(use_alpha)
    return _cache[use_alpha]


def _in_maps(inp):
    ua = int(np.asarray(inp["use_alpha"]))
    x = np.asarray(inp["inputs"], dtype=np.float32).reshape(
        NCORES, C, H * W).astype(ml_dtypes.bfloat16)
    x0 = np.ascontiguousarray(x[:, :, 0:31 * W])
    x1 = np.ascontiguousarray(x[:, :, 28 * W:])
    Alpha = np.ascontiguousarray(np.asarray(inp["Alpha"], dtype=np.float32))
    # [O*C*3*3, T] -> [(i,j), c, t*O + o]
    tmpl = np.asarray(inp["weight_templates"], dtype=np.float32).reshape(
        O, C, 3, 3, T).transpose(2, 3, 1, 4, 0).reshape(9, C, T * O)
    tmpl = np.ascontiguousarray(tmpl).astype(ml_dtypes.bfloat16)
    rwt = np.ascontiguousarray(
        np.asarray(inp["routing_w"], dtype=np.float32).T)
    rb = np.ascontiguousarray(np.asarray(inp["routing_b"], dtype=np.float32))
    bias = np.ascontiguousarray(np.asarray(inp["bias"], dtype=np.float32))

    in_maps = []
    for b in range(NCORES):
        m = {"x0": x0[b], "x1": x1[b], "alpha": Alpha[b], "tmpl": tmpl,
             "rwt": rwt, "rb": rb, "bias": bias}
        if not ua:
            m["mask"] = np.ascontiguousarray(
                np.asarray(inp["mask"][b], dtype=np.int32))
        in_maps.append(m)
    return in_maps


def kernel(inputs, mask, Alpha, weight_templates, routing_w, routing_b, bias,
           use_alpha):
    ua = int(np.asarray(use_alpha))
    nc = _get(ua)
    in_maps = _in_maps(dict(inputs=inputs, mask=mask, Alpha=Alpha,
                            weight_templates=weight_templates,
                            routing_w=routing_w, routing_b=routing_b,
                            bias=bias, use_alpha=use_alpha))
    res = run_bass_kernel_spmd(nc, in_maps, list(range(NCORES)))
    out = np.stack([res.results[b]["out"] for b in range(NCORES)], axis=0)
    out = out[:, :, :NPIX].reshape(NCORES, O, HPAD, WP)[:, :, 1:57, 0:W]
    return np.ascontiguousarray(out)



# revision 2
# speedup vs baseline: 1.1061x; 1.1061x over previous
"""DRConv (dynamic region-aware conv) Trainium2 kernel.

Math (per batch b, all on device):
  x_se  = 0.25*sigmoid(routing_w @ mean_hw(x) + routing_b)        # [G*T]
  Z_t   = conv3x3(x, template_t)       for t in 0..T-1            # [O, H, W]
  U     = x_se contracted with 0.25*softmax(Alpha) over g         # [T, P]
  out   = sum_t Z_t * U_t  + bias                                 # [O, H, W]
which equals the reference
  out = einsum('boghw,bghw->bohw', einsum('bokg,bkhw->boghw', w, patches),
               softmax(Alpha)) + bias
because w = blend(x_se, templates) commutes through the conv: the blend
weights x_se[g,t] and the softmax probs both act per (g, pixel), so the
G-sum and T-sum exchange with the K-contraction.  softmax(Alpha) (or the
one-hot mask when use_alpha=0) is precomputed on the host, so the device
only sees a dense routing-probability plane `ea`.

Sharding: data-parallel over batch B=8, one batch element per NeuronCore.
Templates/routing weights replicated. No collectives.

Device layout (per core):
  pixels live in a 58x57 plane: one pad row top/bottom, ONE pad column
  (a right-pad column doubles as the left neighbor of the next row's
  x=0 pixel, so 57-wide rows give correct 3x3 zero padding);
  pf = (y+1)*57 + x for image pixel (y, x).  The padded plane (plus a
  64-wide zero guard in front) is built on the HOST and shipped as two
  overlapping bf16 bands so no on-device memset/copy is needed.
  conv = 9 shifted matmuls accumulating in PSUM:
    Z[px, (t,o)] += x[c, guard+base+px+delta(i,j)].T @ tmpl[c, (t,o)]
  pixel tiles are the stationary operand (128 px per matmul), so the
  per-pixel routing mix is 8 scalar_tensor_tensor ops on Vector; the
  output stays in [px, O] layout and the host transposes it back.
"""

import ml_dtypes
import numpy as np

import concourse.bass as bass
import concourse.mybir as mybir
from concourse import bacc
from concourse.tile import TileContext
from concourse.bass_utils import run_bass_kernel_spmd

# problem constants
C = 128          # in channels
O = 128          # out channels
H = W = 56
G = 8            # groups
T = 8            # num weight templates
WP = 57          # padded row width (one shared pad column)
NPIX = 58 * WP   # 3306 padded-plane pixels
GUARD = 64       # front zero guard for negative conv shifts
PT0 = WP         # first pixel-tile starts at padded row 1
NT = 25          # 25 tiles of 128 px cover pf [57, 3257) > last valid 3247
PLANE = NT * 128  # 3200 output pixels kept on device
HPW = GUARD + 3392  # host plane width (pf -64 .. 3328)
XBW = 1856       # width of each x band (bands overlap pf [1536,1792))
XB1 = 1536       # pf origin of band B
KSPLIT = 13      # tiles k < KSPLIT read band A, k >= KSPLIT read band B
WARM = 8         # PE warm-up matmuls (p-state ramp during input DMA)
OCHUNK = [0, 7, 13, 19, 23, 25]  # output store chunk boundaries (tiles)
NCORES = 8

_cache = {}

DELTA = [(i - 1) * WP + (j - 1) for i in range(3) for j in range(3)]


def _build():
    f32 = mybir.dt.float32
    bf16 = mybir.dt.bfloat16

    nc = bacc.Bacc("TRN2", target_bir_lowering=False, debug=False,
                   num_devices=NCORES)

    xa_d = nc.dram_tensor("xa", [C, XBW], bf16, kind="ExternalInput")
    xb_d = nc.dram_tensor("xb", [C, XBW], bf16, kind="ExternalInput")
    t_d = [nc.dram_tensor(f"t{c}", [C, 3 * T * O], bf16,
                          kind="ExternalInput") for c in range(3)]
    ea_d = nc.dram_tensor("ea", [G, 3328], bf16, kind="ExternalInput")
    rwt_d = nc.dram_tensor("rwt", [C, G * T], f32, kind="ExternalInput")
    rb_d = nc.dram_tensor("rb", [G * T], f32, kind="ExternalInput")
    bias_d = nc.dram_tensor("bias", [O], f32, kind="ExternalInput")
    out_d = nc.dram_tensor("out", [O, PLANE], f32, kind="ExternalOutput")

    with TileContext(nc) as tc:
        with (
            tc.tile_pool(name="big", bufs=1) as big,
            tc.tile_pool(name="consts", bufs=1) as consts,
            tc.tile_pool(name="acc", bufs=2) as accp,
            tc.tile_pool(name="upool", bufs=3) as upool,
            tc.tile_pool(name="zps", bufs=3, space="PSUM") as zps,
            tc.tile_pool(name="ups", bufs=2, space="PSUM") as ups,
        ):
            # activation-table preload (sigmoid + copy) while DMAs stream
            dummy = consts.tile([1, 2], f32)
            nc.vector.memset(dummy[:], 0.0)
            nc.scalar.activation(dummy[:, 1:2], dummy[:, 0:1],
                                 mybir.ActivationFunctionType.Sigmoid)
            nc.scalar.copy(dummy[:, 0:1], dummy[:, 1:2])

            # PE warm-up source (content irrelevant)
            warm = big.tile([C, 512], bf16)
            nc.vector.memset(warm[:], 0.0)

            # ---- input DMAs, descriptors spread over three queues ----
            tbf = []
            xba = big.tile([C, XBW], bf16)
            xbb = big.tile([C, XBW], bf16)
            nc.sync.dma_start(out=xba[:], in_=xa_d[:])
            tb0 = big.tile([C, 3 * T * O], bf16, name="tb0")
            nc.sync.dma_start(out=tb0[:], in_=t_d[0][:])
            tbf.append(tb0)
            tb1 = big.tile([C, 3 * T * O], bf16, name="tb1")
            nc.sync.dma_start(out=tb1[:], in_=t_d[1][:])
            tbf.append(tb1)
            nc.sync.dma_start(out=xbb[:], in_=xb_d[:])
            tb2 = big.tile([C, 3 * T * O], bf16, name="tb2")
            nc.sync.dma_start(out=tb2[:], in_=t_d[2][:])
            tbf.append(tb2)

            bias_rep = consts.tile([128, O], f32)
            nc.scalar.dma_start(
                out=bias_rep[:],
                in_=bass.AP(tensor=bias_d, offset=0, ap=[[0, 128], [1, O]]))
            rwt = consts.tile([C, G * T], f32)
            nc.scalar.dma_start(out=rwt[:], in_=rwt_d[:])
            rb = consts.tile([G * T, 1], f32)
            nc.scalar.dma_start(out=rb[:], in_=rb_d[:])

            ea = big.tile([G, 3328], bf16)
            nc.gpsimd.dma_start(out=ea[:], in_=ea_d[:])

            # ---- PE warm-up (ramps HAM p-state during the DMAs) ----
            wps = zps.tile([128, 512], f32, tag="zp0", name="warmps")
            for _ in range(WARM):
                nc.tensor.matmul(wps[:], lhsT=warm[:, 0:128], rhs=warm[:])

            # ---- routing GAP (pads are zero, so plain sums work) ----
            xsa = consts.tile([C, 1], f32)
            xsb = consts.tile([C, 1], f32)
            nc.vector.tensor_reduce(
                out=xsa[:], in_=xba[:, 0:GUARD + XB1],
                axis=mybir.AxisListType.X, op=mybir.AluOpType.add)
            nc.vector.tensor_reduce(
                out=xsb[:], in_=xbb[:],
                axis=mybir.AxisListType.X, op=mybir.AluOpType.add)
            xsum = consts.tile([C, 1], f32)
            nc.vector.tensor_add(xsum[:], xsa[:], xsb[:])

            # ---- output plane [px, O], transposed on host ----
            plane = big.tile([128, PLANE], f32)

            zp = {}

            def conv6(k, c, zpk):
                base = PT0 + 128 * k
                for j in range(3):
                    ij = 3 * c + j
                    lo = base + DELTA[ij]
                    if k < KSPLIT:
                        xsl = xba[:, GUARD + lo:GUARD + lo + 128]
                    else:
                        xsl = xbb[:, lo - XB1:lo - XB1 + 128]
                    for h in range(2):
                        nc.tensor.matmul(
                            zpk[h][:],
                            lhsT=xsl,
                            rhs=tbf[c][:, j * 1024 + h * 512:
                                       j * 1024 + (h + 1) * 512],
                            start=(c == 0 and j == 0),
                            stop=(c == 2 and j == 2))

            def alloc_zp(k):
                zp[k] = [zps.tile([128, 512], f32, tag=f"zp{h}",
                                  name=f"zp{h}_{k}") for h in range(2)]

            def up_mm(k):
                up = ups.tile([128, T], f32, tag="up", name=f"up{k}")
                base = PT0 + 128 * k
                nc.tensor.matmul(up[:], lhsT=ea[:, base:base + 128],
                                 rhs=lhsu[:])
                usb = upool.tile([128, T], f32, tag="usb")
                nc.scalar.copy(usb[:], up[:])
                return usb

            def mix(k, usb):
                acc = accp.tile([128, O], f32, tag="acc")
                for t in range(T):
                    h, tq = divmod(t, 4)
                    nc.vector.scalar_tensor_tensor(
                        out=plane[:, k * 128:(k + 1) * 128] if t == T - 1
                        else acc[:],
                        in0=zp[k][h][:, tq * 128:(tq + 1) * 128],
                        scalar=usb[:, t:t + 1],
                        in1=bias_rep[:] if t == 0 else acc[:],
                        op0=mybir.AluOpType.mult,
                        op1=mybir.AluOpType.add)
                del zp[k]

            # interleaved head: tiles 0/1 chunk-major so template-chunk
            # waits overlap ready matmuls; routing chain woven between
            alloc_zp(0)
            alloc_zp(1)
            conv6(0, 0, zp[0])
            conv6(1, 0, zp[1])

            # fc -> sigmoid -> blend weights (PE hits zr ~when xsum lands)
            zr = ups.tile([G * T, 1], f32, tag="up", name="zr")
            nc.tensor.matmul(zr[:], lhsT=rwt[:], rhs=xsum[:])
            xse = consts.tile([G * T, 1], bf16)
            nc.scalar.activation(xse[:], zr[:],
                                 mybir.ActivationFunctionType.Sigmoid,
                                 bias=rb[:], scale=1.0 / (H * W))
            # [64,1] -> [8,8] partition/free reshape is a tiny DMA;
            # the 2/T scale is folded into the host-side ea plane
            lhsu = consts.tile([G, T], bf16)
            nc.sync.dma_start(out=lhsu[:], in_=xse[:])

            conv6(0, 1, zp[0])
            conv6(1, 1, zp[1])
            conv6(0, 2, zp[0])
            usb0 = up_mm(0)
            conv6(1, 2, zp[1])
            usb1 = up_mm(1)
            mix(0, usb0)

            alloc_zp(2)
            usb2 = up_mm(2)
            for c in range(3):
                conv6(2, c, zp[2])
            mix(1, usb1)
            mix(2, usb2)

            nchunk = 1
            for k in range(3, NT):
                alloc_zp(k)
                usb = up_mm(k)
                for c in range(3):
                    conv6(k, c, zp[k])
                mix(k, usb)
                if k + 1 == OCHUNK[nchunk]:
                    lo, hi = OCHUNK[nchunk - 1] * 128, OCHUNK[nchunk] * 128
                    nc.sync.dma_start(out=out_d[:, lo:hi],
                                      in_=plane[:, lo:hi])
                    nchunk += 1

    nc.compile()
    return nc


def _get():
    if "nc" not in _cache:
        _cache["nc"] = _build()
    return _cache["nc"]


def _in_maps(inp):
    ua = int(np.asarray(inp["use_alpha"]))
    x = np.asarray(inp["inputs"], dtype=np.float32)
    # host-padded image plane: pf = (y+1)*57 + x, 64-wide front guard
    hp = np.zeros((NCORES, C, HPW), dtype=ml_dtypes.bfloat16)
    hp[:, :, GUARD + WP:GUARD + WP + H * WP].reshape(
        NCORES, C, H, WP)[:, :, :, 0:W] = x.reshape(NCORES, C, H, W)
    xa = hp[:, :, 0:XBW]
    xb = hp[:, :, GUARD + XB1:GUARD + XB1 + XBW]

    # routing probabilities (softmax or one-hot), 2/T scale folded in
    if ua:
        a = np.asarray(inp["Alpha"], dtype=np.float32)
        e = np.exp(a - a.max(axis=1, keepdims=True))
        probs = e / e.sum(axis=1, keepdims=True)
    else:
        m = np.asarray(inp["mask"])
        probs = (m[:, None, :, :] == np.arange(G)[None, :, None, None])
        probs = probs.astype(np.float32)
    probs *= 2.0 / T
    eap = np.zeros((NCORES, G, 3328), dtype=ml_dtypes.bfloat16)
    eap[:, :, WP:WP + H * WP].reshape(
        NCORES, G, H, WP)[:, :, :, 0:W] = probs

    # [O*C*3*3, T] -> 3 chunks of [C, 3*(t*O+o)], ij-major
    t9 = np.asarray(inp["weight_templates"], dtype=np.float32).reshape(
        O, C, 3, 3, T).transpose(2, 3, 1, 4, 0).reshape(9, C, T * O)
    tch = [np.ascontiguousarray(
        t9[3 * c:3 * c + 3].transpose(1, 0, 2).reshape(C, 3 * T * O)
    ).astype(ml_dtypes.bfloat16) for c in range(3)]
    rwt = np.ascontiguousarray(
        np.asarray(inp["routing_w"], dtype=np.float32).T)
    rb = np.ascontiguousarray(np.asarray(inp["routing_b"], dtype=np.float32))
    bias = np.ascontiguousarray(np.asarray(inp["bias"], dtype=np.float32))

    in_maps = []
    for b in range(NCORES):
        in_maps.append({
            "xa": np.ascontiguousarray(xa[b]),
            "xb": np.ascontiguousarray(xb[b]),
            "t0": tch[0], "t1": tch[1], "t2": tch[2],
            "ea": np.ascontiguousarray(eap[b]),
            "rwt": rwt, "rb": rb, "bias": bias,
        })
    return in_maps


def kernel(inputs, mask, Alpha, weight_templates, routing_w, routing_b, bias,
           use_alpha):
    nc = _get()
    in_maps = _in_maps(dict(inputs=inputs, mask=mask, Alpha=Alpha,
                            weight_templates=weight_templates,
                            routing_w=routing_w, routing_b=routing_b,
                            bias=bias, use_alpha=use_alpha))
    res = run_bass_kernel_spmd(nc, in_maps, list(range(NCORES)))
    arr = np.stack([res.results[b]["out"] for b in range(NCORES)], axis=0)
    # [b, px_in_tile, (k, o)] -> [b, o, pf-57] -> [b, O, H, W]
    out = arr.reshape(NCORES, 128, NT, O).transpose(0, 3, 2, 1).reshape(
        NCORES, O, PLANE)[:, :, 0:H * WP].reshape(NCORES, O, H, WP)[
        :, :, :, 0:W]
    return np.ascontiguousarray(out.astype(np.float32))
